# revision 64
# baseline (speedup 1.0000x reference)
"""Causal cross-attention kernel for 8 TRN2 NeuronCores.

Sharding: data-parallel over batch (B=2) x tensor-parallel over head
groups (16 heads -> 4 groups of 4). Core c handles batch c//4, heads
[4*(c%4), 4*(c%4)+4). Each core computes its partial output projection
(w_out rows for its heads); the host sums the 4 partials per batch
(the "all-reduce"), adds b_out, and fixes the fully-masked row 0.

Precision: all matmuls run 16-bit operands (1 PE cycle/row). The Q/K
path is fp16 (exp() amplifies logit rounding; fp16 ~5e-3 rel err).
P=exp(scores) can reach e^51 so it must be bf16 (fp16 max 65504), which
forces v1 (the other AV operand) to bf16 too. PSUM accumulation is f32.

Schedule (v4): the PE-HAM clock gate re-throttles the PE array to half
clock whenever its pipeline has idle windows; the exp-dependent
attention inner loop alone cannot keep it fully busy. So the kernel is
one flat software-pipelined stream of attention "superpairs" (2 heads x
2 z-blocks) with projection / output-projection units interleaved
between them as PE filler:

  superpair sp: scores(sp) -> mask(sp) -> exp(sp) [ACT] -> AV(sp-2)

Scores for the head pair run concurrently in the PE array (head hh=0
at rows 0-63, hh=1 at rows 64-127 via tile_position row groups). Each
head's scores live in a [128, 2, 512] PSUM tile from a shared 3-slot
pool (6 banks) that the projection units also draw from; pav
accumulators take the remaining 2 banks. AV trails exp by two
superpairs so the ACT engine never gates the PE.

Softmax denominators (row D of pav, via the ones-column of v1) are
inverted as exp(-ln(x)) on ACT, PE-broadcast to 64 partitions, and
multiplied into aoT on DVE; the whole tail is deferred into the next
m-group's attention so the PE never waits on it.
"""

import numpy as np
import concourse.bass as bass
import concourse.mybir as mybir
import concourse.tile as tile
from concourse.bass_utils import run_bass_kernel_spmd

B, S, F, H = 2, 2048, 1024, 16
NCORES = 8
HG = 4          # head groups (tensor-parallel degree per batch)
HPC = H // HG   # heads per core = 4
D = F // H      # head dim = 64
CW = HPC * D    # per-core projection width = 256
MASK_VAL = 1.0e12
SC = 512        # s-chunk
NSC = S // SC   # 4

f32 = mybir.dt.float32
f16 = mybir.dt.float16
bf16 = mybir.dt.bfloat16

# Walrus encodes at most 1 sync wait on most TRN2 instructions; Tile can
# attach several. Redistribute excess waits onto preceding same-engine NOPs.


def _split_excess_waits(nc):
    for fn in nc.m.functions:
        for bb in fn.blocks:
            insts = list(bb.instructions)
            out = []
            changed = False
            for inst in insts:
                si = inst.sync_info
                waits = list(si.on_wait) if si is not None else []
                if len(waits) > 1:
                    changed = True
                    inst.sync_info = mybir.SyncInfo(
                        on_update=list(si.on_update), on_wait=waits[-1:]
                    )
                    for idx, w in enumerate(waits[:-1]):
                        nop = mybir.InstNoOp(name=f"{inst.name}-wsplit{idx}")
                        nop.engine = inst.engine
                        nop.sync_info = mybir.SyncInfo(on_update=[], on_wait=[w])
                        out.append(nop)
                out.append(inst)
            if changed:
                bb.instructions = out


def _build():
    nc = bass.Bass()
    # x inputs are transposed [F, S]; weights come host-prepacked into one
    # [128, 4*2048] blob (sections: wq, wk, wv k-major; wo m-major) so each
    # DMA row is 16 KB contiguous — the [F, CW] layouts would give 512B
    # lines at ~1/4 of DMA peak.
    xf_d = nc.declare_dram_parameter("xf", [F, S], f16, isOutput=False)
    xt_d = nc.declare_dram_parameter("xt", [F, S], f16, isOutput=False)
    wpk_d = nc.declare_dram_parameter("wpk", [128, 4, 8 * CW], f16, isOutput=False)
    bq_d = nc.declare_dram_parameter("bq", [CW, 1], f32, isOutput=False)
    bk_d = nc.declare_dram_parameter("bk", [CW, 1], f32, isOutput=False)
    msk_d = nc.declare_dram_parameter("msk", [128, 128], bf16, isOutput=False)
    out_d = nc.declare_dram_parameter("out", [S, F], f16, isOutput=True)

    with tile.TileContext(nc) as tc:
        with (
            tc.tile_pool(name="const", bufs=1) as cpool,
            tc.tile_pool(name="xf", bufs=2) as xfpool,
            tc.tile_pool(name="xt", bufs=2) as xtpool,
            tc.tile_pool(name="pbuf", bufs=8) as ppool,
            tc.tile_pool(name="work", bufs=4) as wpool,
            tc.tile_pool(name="outst", bufs=4) as opool,
            tc.tile_pool(name="ps_u", bufs=3, space="PSUM") as ps_u,
            tc.tile_pool(name="ps_av", bufs=2, space="PSUM") as ps_av,
        ):
            # ---- persistent tiles ----
            wpk = cpool.tile([128, 4, 8, CW], f16)
            wq = wpk[:, 0, :, :]
            wk = wpk[:, 1, :, :]
            wv = wpk[:, 2, :, :]

            # wo section is m-major flat: element (m*F + c) at [k=idx//CW,
            # c%CW]; slice [m, fo*SC:(fo+1)*SC] = two whole k-subrows.
            def wo_sl(m, fo):
                return wpk[:, 3, 4 * m + 2 * fo : 4 * m + 2 * fo + 2, :]
            bq = cpool.tile([128, 2, 1], f32)
            bk = cpool.tile([128, 2, 1], f32)
            msk = cpool.tile([128, 128], bf16)
            ones_b = cpool.tile([1, 128], bf16)
            nbias = cpool.tile([128, 1], f32)
            qT = cpool.tile([128, 2, S], f16)
            kT = cpool.tile([128, 2, S], f16)
            v1 = cpool.tile([128, S // 128, HPC, D + 1], bf16)
            aoT = cpool.tile([128, 2, S], f16)

            # x tensors load in chunk PAIRS ([128, k, 1024] per k-slice =
            # 2 KB DRAM lines -> ~2x the DMA packet efficiency of 1 KB).
            xfile, xtile = {}, {}

            def emit_dma(pair):
                s0 = pair * 2 * SC
                xf = xfpool.tile([128, 8, 2 * SC], f16, tag="xf", name=f"xf{pair}")
                xt = xtpool.tile([128, 8, 2 * SC], f16, tag="xt", name=f"xt{pair}")
                for sc in (2 * pair, 2 * pair + 1):
                    o = (sc % 2) * SC
                    xfile[sc] = xf[:, :, o : o + SC]
                    xtile[sc] = xt[:, :, o : o + SC]
                for k in range(8):
                    nc.sync.dma_start(
                        out=xf[:, k, :],
                        in_=xf_d[128 * k : 128 * (k + 1), s0 : s0 + 2 * SC],
                    )
                    nc.sync.dma_start(
                        out=xt[:, k, :],
                        in_=xt_d[128 * k : 128 * (k + 1), s0 : s0 + 2 * SC],
                    )

            # startup: weight-blob sections interleaved with the first x
            # loads so the first q-projection matmul starts after ~0.5 MB.
            nc.sync.dma_start(out=bq[:], in_=bq_d[:].rearrange("(m p) c -> p m c", p=128))
            nc.sync.dma_start(out=bk[:], in_=bk_d[:].rearrange("(m p) c -> p m c", p=128))
            nc.sync.dma_start(out=msk[:], in_=msk_d[:])
            nc.vector.memset(ones_b[:], 1.0)
            nc.vector.memset(nbias[:], -12.0)
            # ones column of v1 (col D) -> row D of pav = softmax denominator
            nc.vector.memset(v1[:, :, :, D : D + 1], 1.0)
            xf0 = xfpool.tile([128, 8, 2 * SC], f16, tag="xf", name="xf0")
            xt0 = xtpool.tile([128, 8, 2 * SC], f16, tag="xt", name="xt0")
            for sc in (0, 1):
                o = sc * SC
                xfile[sc] = xf0[:, :, o : o + SC]
                xtile[sc] = xt0[:, :, o : o + SC]
            wpk0 = wpk_d[:, 0, :].rearrange("p (k c) -> p k c", c=CW)
            nc.sync.dma_start(out=wpk[:, 0, 0:4, :], in_=wpk0[:, 0:4, :])
            nc.sync.dma_start(out=xf0[:, 0, :], in_=xf_d[0:128, 0 : 2 * SC])
            nc.sync.dma_start(out=wpk[:, 0, 4:8, :], in_=wpk0[:, 4:8, :])
            for k in range(1, 8):
                nc.sync.dma_start(out=xf0[:, k, :], in_=xf_d[128 * k : 128 * (k + 1), 0 : 2 * SC])
            nc.sync.dma_start(out=wpk[:, 1, :, :], in_=wpk_d[:, 1, :].rearrange("p (k c) -> p k c", c=CW))
            for k in range(8):
                nc.sync.dma_start(out=xt0[:, k, :], in_=xt_d[128 * k : 128 * (k + 1), 0 : 2 * SC])
            nc.sync.dma_start(out=wpk[:, 2, :, :], in_=wpk_d[:, 2, :].rearrange("p (k c) -> p k c", c=CW))
            nc.sync.dma_start(out=wpk[:, 3, :, :], in_=wpk_d[:, 3, :].rearrange("p (k c) -> p k c", c=CW))

            # ---------- projection / out-projection units ----------
            # Each unit takes one [128, 2, SC] slot from the shared PSUM
            # pool (also used by attention score tiles) and is emitted
            # between attention superpairs as PE filler.

            def unit_qk(sc, which):
                s0 = sc * SC
                w, bias, dstT = (
                    (wq, bq, qT) if which == "q" else (wk, bk, kT)
                )
                x = xfile[sc] if which == "q" else xtile[sc]
                pu = ps_u.tile([128, 2, SC], f32, tag="u", name=f"p{which}{sc}")
                for m in range(2):
                    for k in range(8):
                        nc.tensor.matmul(
                            pu[:, m, :],
                            w[:, k, m * 128 : (m + 1) * 128],
                            x[:, k, :],
                            start=(k == 0),
                            stop=(k == 7),
                        )
                for m in range(2):
                    nc.vector.tensor_scalar_add(
                        dstT[:, m, s0 : s0 + SC], pu[:, m, :], bias[:, m, :]
                    )

            def unit_v(sc, vp):
                # NOTE: b_kv[F:] (the v bias) is NOT added here — softmax
                # weights sum to 1, so its contribution to the output is the
                # constant vector bv @ w_out, which the host adds.
                pu = ps_u.tile([128, 2, SC], f32, tag="u", name=f"pv{sc}_{vp}")
                for j in range(2):
                    zz = 2 * vp + j
                    for k in range(8):
                        nc.tensor.matmul(
                            pu[:, j, 0:CW],
                            xtile[sc][:, k, zz * 128 : (zz + 1) * 128],
                            wv[:, k, :],
                            start=(k == 0),
                            stop=(k == 7),
                        )
                for j in range(2):
                    zc = sc * 4 + 2 * vp + j
                    nc.vector.tensor_copy(
                        v1[:, zc, :, 0:D],
                        pu[:, j, 0:CW].rearrange("p (h d) -> p h d", h=HPC),
                    )

            def unit_out(sc, so):
                s0o = sc * SC + so * 128
                pu = ps_u.tile([128, 2, SC], f32, tag="u", name=f"po{sc}_{so}")
                for fo in range(2):
                    for m in range(2):
                        nc.tensor.matmul(
                            pu[:, fo, :],
                            aoT[:, m, s0o : s0o + 128],
                            wo_sl(m, fo),
                            start=(m == 0),
                            stop=(m == 1),
                        )
                ost = opool.tile([128, F], f16, tag="ost")
                nc.vector.tensor_copy(
                    ost[:].rearrange("p (a b) -> p a b", a=2), pu[:, :, :]
                )
                nc.sync.dma_start(out=out_d[s0o : s0o + 128, :], in_=ost[:])

            # chunk-0 projections as a dense prologue
            unit_qk(0, "q")
            unit_qk(0, "k")
            unit_v(0, 0)
            unit_v(0, 1)

            # ---------- flat attention stream ----------
            # pending AVs: deque of closures; AV(sp) is emitted after
            # exp(sp+2) so ACT has two superpairs of slack. Norm tails are
            # due-scheduled two superpairs after their group's last AV.
            av_q = []
            AV_LAG = 2
            # First superpair slot for out-proj filler: the previous
            # chunk's deferred norm flushes at sp AV_LAG+1, and flush runs
            # before fillers within an iteration, so AV_LAG+1 is safe.
            FILL0 = AV_LAG + 1
            pending_norm = []  # (due_gsp, closure)
            gsp = [0]

            def flush_norm(force=False):
                while pending_norm and (force or pending_norm[0][0] <= gsp[0]):
                    pending_norm.pop(0)[1]()

            def drain_av(keep):
                while len(av_q) > keep:
                    av_q.pop(0)()

            for sc in range(NSC):
                s0 = sc * SC
                npair = 2 * (sc + 1)
                nsp = 2 * npair

                # Filler budget per chunk is matched to that chunk's ACT
                # slack: chunk3 (longest attention, no next-chunk proj) gets
                # two out-projection chunks; out(3) runs after the loop.
                # Projection units for the next chunk can fill anywhere; the
                # out-projection units read aoT, so they must wait until the
                # previous chunk's deferred norm has been flushed (FILL0) —
                # Tile tracks access history incrementally in emission order.
                filler = []
                if sc + 1 < NSC:
                    if sc == 0:
                        filler.append(lambda: emit_dma(1))
                    filler.append(lambda sc=sc: unit_qk(sc + 1, "q"))
                    filler.append(lambda sc=sc: unit_qk(sc + 1, "k"))
                    filler.append(lambda sc=sc: unit_v(sc + 1, 0))
                    filler.append(lambda sc=sc: unit_v(sc + 1, 1))
                late_filler = []
                out_chunks = {2: [0], 3: [1, 2]}.get(sc, [])
                for oc in out_chunks:
                    for so in range(SC // 128):
                        late_filler.append(lambda oc=oc, so=so: unit_out(oc, so))
                import collections as _c

                fill_at = _c.defaultdict(list)
                for u in range(len(filler)):
                    fill_at[min(nsp - 1, 1 + int(u * (nsp - 1) / len(filler)))].append(
                        ("f", u)
                    )
                f0 = min(FILL0, max(0, nsp - 1))
                for u in range(len(late_filler)):
                    fill_at[
                        min(nsp - 1, f0 + int(u * (nsp - f0) / len(late_filler)))
                    ].append(("l", u))

                # dz: first valid s-column of z-block (2p+j); scores/AV skip
                # columns left of it (fully masked). Only a [128,128]
                # staircase band at [dz, dz+128) mixes masked/unmasked.
                def dz_of(p, j, sc=sc):
                    if p < 2 * sc:
                        return -1
                    return 256 * (p - 2 * sc) + 128 * j

                sp_idx = 0
                for m in range(2):
                    pav = [
                        ps_av.tile(
                            [D + 1, SC], f32, tag="pav", name=f"pav{sc}_{m}_{hh}"
                        )
                        for hh in range(2)
                    ]

                    def make_av(p, pt, pav=pav, m=m, npair=npair, dz=dz_of):
                        def emit():
                            for hh in range(2):
                                for j in range(2):
                                    d0 = max(0, dz(p, j))
                                    nc.tensor.matmul(
                                        pav[hh][:, d0:SC],
                                        v1[:, 2 * p + j, 2 * m + hh, :],
                                        pt[hh][:, j, d0:SC],
                                        start=(p == 0 and j == 0),
                                        stop=(p == npair - 1 and j == 1),
                                    )

                        return emit

                    def make_evac(pav=pav, m=m, sc=sc):
                        # Evacuate pav to SBUF right after the group's last
                        # AV: frees the 2 pav banks for the next group ~1us
                        # after the group ends, instead of after the whole
                        # normalization chain.
                        avs = [
                            wpool.tile(
                                [D + 1, SC],
                                f32,
                                tag="avs",
                                name=f"avs{sc}_{m}_{hh}",
                            )
                            for hh in range(2)
                        ]

                        def emit():
                            for hh in range(2):
                                nc.vector.tensor_copy(avs[hh][:, :], pav[hh][:, :])

                        return avs, emit

                    def make_recip(avs, m=m, sc=sc):
                        # 1/den as exp(-ln(den)) on ACT (bf16 out); hoisted
                        # on the saturated ACT queue so it runs as soon as
                        # the denominators land in SBUF.
                        rcbs = [
                            wpool.tile(
                                [1, SC], bf16, tag="rcb", name=f"rcb{sc}_{m}_{hh}"
                            )
                            for hh in range(2)
                        ]

                        def emit():
                            for hh in range(2):
                                lg = wpool.tile(
                                    [1, SC], f32, tag="lg", name=f"lg{sc}_{m}_{hh}"
                                )
                                with tc.high_priority(offset=25):
                                    nc.scalar.activation(
                                        lg[:, :],
                                        avs[hh][D : D + 1, :],
                                        mybir.ActivationFunctionType.Ln,
                                    )
                                    nc.scalar.activation(
                                        rcbs[hh][:, :],
                                        lg[:, :],
                                        mybir.ActivationFunctionType.Exp,
                                        scale=-1.0,
                                    )

                        return rcbs, emit

                    def make_norm(avs, rcbs, m=m, s0=s0, sc=sc):
                        def emit():
                            # PE broadcast of 1/den via ones lhsT, scale into
                            # aoT on DVE. den==0 (row 0) -> garbage only in
                            # column s=0, overwritten by the host.
                            for hh in range(2):
                                po = hh * D
                                pb = ps_u.tile(
                                    [D, SC], f32, tag="u", name=f"pb{sc}_{m}_{hh}"
                                )
                                nc.tensor.matmul(
                                    pb[:, :],
                                    ones_b[:, :D],
                                    rcbs[hh][:, :],
                                    start=True,
                                    stop=True,
                                )
                                sb = wpool.tile(
                                    [D, SC], bf16, tag="sb", name=f"sb{sc}_{m}_{hh}"
                                )
                                nc.vector.tensor_copy(sb[:, :], pb[:, :])
                                nc.vector.tensor_tensor(
                                    out=aoT[po : po + D, m, s0 : s0 + SC],
                                    in0=avs[hh][0:D, :],
                                    in1=sb[:, :],
                                    op=mybir.AluOpType.mult,
                                )

                        return emit

                    for p in range(npair):
                        psp = []
                        pt = []
                        for hh in range(2):
                            po = hh * D
                            ps_h = ps_u.tile(
                                [128, 2, SC],
                                f32,
                                tag="u",
                                name=f"ps{sc}_{m}_{p}_{hh}",
                            )
                            psp.append(ps_h)
                            for j in range(2):
                                z0 = (2 * p + j) * 128
                                d0 = max(0, dz_of(p, j))
                                nc.tensor.matmul(
                                    ps_h[:, j, d0:SC],
                                    kT[po : po + D, m, z0 : z0 + 128],
                                    qT[po : po + D, m, s0 + d0 : s0 + SC],
                                    start=True,
                                    stop=True,
                                )
                        # -12 shift: softmax-invariant, keeps den=sum(exp) in
                        # [2^-46, 2^58] — the ACT Ln table used for 1/den
                        # breaks beyond ~2^63 (logits ~51 -> den ~2^75).
                        # Deep-diagonal pairs (d0 >= 256) are cheaper as two
                        # sliced exps than one full-width one.
                        for hh in range(2):
                            pt_h = ppool.tile(
                                [128, 2, SC],
                                bf16,
                                tag="p",
                                name=f"pt{sc}_{m}_{p}_{hh}",
                            )
                            pt.append(pt_h)
                            if dz_of(p, 0) >= 256:
                                for j in range(2):
                                    d0 = dz_of(p, j)
                                    nc.scalar.activation(
                                        pt_h[:, j, d0:SC],
                                        psp[hh][:, j, d0:SC],
                                        mybir.ActivationFunctionType.Exp,
                                        bias=nbias[:, :],
                                    )
                            else:
                                nc.scalar.activation(
                                    pt_h[:, :, :],
                                    psp[hh][:, :, :],
                                    mybir.ActivationFunctionType.Exp,
                                    bias=nbias[:, :],
                                )
                        for hh in range(2):
                            for j in range(2):
                                d0 = dz_of(p, j)
                                if d0 >= 0:
                                    nc.vector.tensor_tensor(
                                        out=pt[hh][:, j, d0 : d0 + 128],
                                        in0=pt[hh][:, j, d0 : d0 + 128],
                                        in1=msk[:, :],
                                        op=mybir.AluOpType.mult,
                                    )
                        av_q.append(make_av(p, pt))
                        drain_av(AV_LAG)
                        flush_norm()
                        for kind, u in fill_at.get(sp_idx, ()):
                            (filler if kind == "f" else late_filler)[u]()
                        sp_idx += 1
                        gsp[0] += 1
                    # The evac depends on this group's last AV, which drains
                    # AV_LAG-1 superpairs from now (drain runs before flush in
                    # the loop). Both it and the norm MUST be emitted before
                    # anything that reads aoT or reuses the pav banks: Tile
                    # tracks access history incrementally in emission order,
                    # so a reader emitted after an overwriting write is NOT
                    # protected.
                    avs, evac = make_evac()
                    rcbs, recip = make_recip(avs)
                    pending_norm.append((gsp[0] + AV_LAG - 1, evac))
                    pending_norm.append((gsp[0] + AV_LAG, recip))
                    pending_norm.append((gsp[0] + AV_LAG + 1, make_norm(avs, rcbs)))

            drain_av(0)
            flush_norm(force=True)
            for so in range(SC // 128):
                unit_out(NSC - 1, so)

    _split_excess_waits(nc)
    return nc


_CACHE = {}


def _get_nc():
    if "nc" not in _CACHE:
        _CACHE["nc"] = _build()
    return _CACHE["nc"]


def _ensure_ntff_hook():
    """The agent image's antenv lacks axon_hooks, so run_bass_kernel_spmd's
    trace path can't import it. Synthesize the module and install the
    ctypes NTFF hook from trn_agent_boot (same thing boot() would do)."""
    import sys
    import types

    if "antenv.axon_hooks" not in sys.modules:
        mod = types.ModuleType("antenv.axon_hooks")
        holder = [None]
        mod.set_axon_ntff_profile_hook = lambda h: holder.__setitem__(0, h)
        mod.get_axon_ntff_profile_hook = lambda: holder[0]
        sys.modules["antenv.axon_hooks"] = mod
        import antenv

        antenv.axon_hooks = mod
    import antenv.axon_hooks as ah

    if ah.get_axon_ntff_profile_hook() is None:
        try:
            from trn_agent_boot.trn_boot import _ntff_profile_via_ctypes

            ah.set_axon_ntff_profile_hook(
                _ntff_profile_via_ctypes("/opt/axon/libaxon_pjrt.so")
            )
        except Exception:
            pass


def _host_mask():
    # The causal band of every diagonal score block (z0 = s0 + dz) reduced to
    # its mixed [dz, dz+128) columns is the same staircase: element
    # (z = z0 + i, s = s0 + dz + c) is masked iff s <= z iff c <= i. Applied
    # multiplicatively to exp(scores) in bf16.
    i = np.arange(128)[:, None]
    c = np.arange(128)[None, :]
    import jax.numpy as _jnp
    m = np.where(c <= i, 0.0, 1.0).astype(np.float32)
    return np.asarray(_jnp.asarray(m, dtype=_jnp.bfloat16))


def kernel(attend_from, attend_to, w_q, b_q, w_kv, b_kv, w_out, b_out, _trace=False):
    attend_from = np.asarray(attend_from, dtype=np.float32)
    attend_to = np.asarray(attend_to, dtype=np.float32)
    w_q = np.asarray(w_q, dtype=np.float32)
    b_q = np.asarray(b_q, dtype=np.float32)
    w_kv = np.asarray(w_kv, dtype=np.float32)
    b_kv = np.asarray(b_kv, dtype=np.float32)
    w_out = np.asarray(w_out, dtype=np.float32)
    b_out = np.asarray(b_out, dtype=np.float32)

    msk = _host_mask()
    xT = [attend_from[b].T.astype(np.float16) for b in range(B)]
    xTt = [attend_to[b].T.astype(np.float16) for b in range(B)]

    def kmajor(w):
        # [F, CW] -> [128, F//128 * CW]: row p = concat_k w[k*128+p, :]
        return np.ascontiguousarray(
            w.reshape(8, 128, CW).transpose(1, 0, 2).reshape(128, 8 * CW)
        )

    in_maps = []
    for c in range(NCORES):
        b, hg = divmod(c, HG)
        cols = slice(hg * CW, (hg + 1) * CW)
        # wo section: [CW, F] m-major -> [128, 2*F]: row p =
        # concat_m w_out[m*128+p, :]
        wo_pk = (
            w_out[cols, :]
            .reshape(2, 128, F)
            .transpose(1, 0, 2)
            .reshape(128, 2 * F)
        )
        wpk = np.stack(
            [
                kmajor(w_q[:, cols]),
                kmajor(w_kv[:, cols]),
                kmajor(w_kv[:, F:][:, cols]),
                wo_pk,
            ],
            axis=1,
        ).astype(np.float16)
        in_maps.append(
            {
                "xf": xT[b],
                "xt": xTt[b],
                "wpk": np.ascontiguousarray(wpk),
                "bq": np.ascontiguousarray(b_q[cols].reshape(CW, 1)),
                "bk": np.ascontiguousarray(b_kv[cols].reshape(CW, 1)),
                "msk": msk,
                "out": np.zeros((S, F), np.float16),
            }
        )

    nc = _get_nc()
    if _trace:
        _ensure_ntff_hook()
    res = run_bass_kernel_spmd(nc, in_maps, list(range(NCORES)), trace=_trace)

    out = np.zeros((B, S, F), np.float64)
    for c in range(NCORES):
        b = c // HG
        out[b] += res.results[c]["out"].astype(np.float64)
    # v-bias contribution: softmax rows sum to 1, so it collapses to the
    # constant vector b_v @ w_out (the kernel omits it)
    out += (
        b_out.astype(np.float64)
        + b_kv[F:].astype(np.float64) @ w_out.astype(np.float64)
    )[None, None, :]

    # Row 0 of the reference is fully masked -> softmax is exactly uniform
    # over all Z positions (the -1e12 shift absorbs the logits in f32);
    # compute it directly on the host.
    w_v = w_kv[:, F:].astype(np.float64)
    for b in range(B):
        val_mean = attend_to[b].astype(np.float64).mean(axis=0) @ w_v + b_kv[
            F:
        ].astype(np.float64)
        out[b, 0, :] = val_mean @ w_out.astype(np.float64) + b_out.astype(np.float64)

    if _trace:
        kernel._last_result = res
    return out.astype(np.float32)


# revision 65
# speedup vs baseline: 1.0775x; 1.0775x over previous
"""Causal cross-attention kernel for 8 TRN2 NeuronCores.

Sharding: data-parallel over batch (B=2) x tensor-parallel over head
groups (16 heads -> 4 groups of 4). Core c handles batch c//4, heads
[4*(c%4), 4*(c%4)+4). Each core computes its partial output projection
(w_out rows for its heads); the host sums the 4 partials per batch
(the "all-reduce"), adds b_out, and fixes the fully-masked row 0.

Precision: all matmuls run 16-bit operands (1 PE cycle/row). The Q/K
path is fp16 (exp() amplifies logit rounding; fp16 ~5e-3 rel err).
P=exp(scores) can reach e^51 so it must be bf16 (fp16 max 65504), which
forces v1 (the other AV operand) to bf16 too. PSUM accumulation is f32.

Schedule (v4): the PE-HAM clock gate re-throttles the PE array to half
clock whenever its pipeline has idle windows; the exp-dependent
attention inner loop alone cannot keep it fully busy. So the kernel is
one flat software-pipelined stream of attention "superpairs" (2 heads x
2 z-blocks) with projection / output-projection units interleaved
between them as PE filler:

  superpair sp: scores(sp) -> mask(sp) -> exp(sp) [ACT] -> AV(sp-2)

Scores for the head pair run concurrently in the PE array (head hh=0
at rows 0-63, hh=1 at rows 64-127 via tile_position row groups). Each
head's scores live in a [128, 2, 512] PSUM tile from a shared 3-slot
pool (6 banks) that the projection units also draw from; pav
accumulators take the remaining 2 banks. AV trails exp by two
superpairs so the ACT engine never gates the PE.

Softmax denominators (row D of pav, via the ones-column of v1) are
inverted as exp(-ln(x)) on ACT, PE-broadcast to 64 partitions, and
multiplied into aoT on DVE; the whole tail is deferred into the next
m-group's attention so the PE never waits on it.
"""

import numpy as np
import concourse.bass as bass
import concourse.mybir as mybir
import concourse.tile as tile
from concourse.bass_utils import run_bass_kernel_spmd

B, S, F, H = 2, 2048, 1024, 16
NCORES = 8
HG = 4          # head groups (tensor-parallel degree per batch)
HPC = H // HG   # heads per core = 4
D = F // H      # head dim = 64
CW = HPC * D    # per-core projection width = 256
MASK_VAL = 1.0e12
SC = 512        # s-chunk
NSC = S // SC   # 4

f32 = mybir.dt.float32
f16 = mybir.dt.float16
bf16 = mybir.dt.bfloat16

# Walrus encodes at most 1 sync wait on most TRN2 instructions; Tile can
# attach several. Redistribute excess waits onto preceding same-engine NOPs.


def _split_excess_waits(nc):
    for fn in nc.m.functions:
        for bb in fn.blocks:
            insts = list(bb.instructions)
            out = []
            changed = False
            for inst in insts:
                si = inst.sync_info
                waits = list(si.on_wait) if si is not None else []
                if len(waits) > 1:
                    changed = True
                    inst.sync_info = mybir.SyncInfo(
                        on_update=list(si.on_update), on_wait=waits[-1:]
                    )
                    for idx, w in enumerate(waits[:-1]):
                        nop = mybir.InstNoOp(name=f"{inst.name}-wsplit{idx}")
                        nop.engine = inst.engine
                        nop.sync_info = mybir.SyncInfo(on_update=[], on_wait=[w])
                        out.append(nop)
                out.append(inst)
            if changed:
                bb.instructions = out


def _build():
    nc = bass.Bass()
    # x inputs are transposed [F, S]; weights come host-prepacked into one
    # [128, 4*2048] blob (sections: wq, wk, wv k-major; wo m-major) so each
    # DMA row is 16 KB contiguous — the [F, CW] layouts would give 512B
    # lines at ~1/4 of DMA peak.
    xf_d = nc.declare_dram_parameter("xf", [F, S], f16, isOutput=False)
    xt_d = nc.declare_dram_parameter("xt", [F, S], f16, isOutput=False)
    wpk_d = nc.declare_dram_parameter("wpk", [128, 4, 8 * CW], f16, isOutput=False)
    bq_d = nc.declare_dram_parameter("bq", [CW, 1], f32, isOutput=False)
    bk_d = nc.declare_dram_parameter("bk", [CW, 1], f32, isOutput=False)
    msk_d = nc.declare_dram_parameter("msk", [128, 128], bf16, isOutput=False)
    out_d = nc.declare_dram_parameter("out", [S, F], f16, isOutput=True)

    with tile.TileContext(nc) as tc:
        with (
            tc.tile_pool(name="const", bufs=1) as cpool,
            tc.tile_pool(name="xf", bufs=2) as xfpool,
            tc.tile_pool(name="xt", bufs=2) as xtpool,
            tc.tile_pool(name="pbuf", bufs=8) as ppool,
            tc.tile_pool(name="work", bufs=4) as wpool,
            tc.tile_pool(name="outst", bufs=4) as opool,
            tc.tile_pool(name="ps_u", bufs=3, space="PSUM") as ps_u,
            tc.tile_pool(name="ps_av", bufs=2, space="PSUM") as ps_av,
        ):
            # ---- persistent tiles ----
            wpk = cpool.tile([128, 4, 8, CW], f16)
            wq = wpk[:, 0, :, :]
            wk = wpk[:, 1, :, :]
            wv = wpk[:, 2, :, :]

            # wo section is m-major flat: element (m*F + c) at [k=idx//CW,
            # c%CW]; slice [m, fo*SC:(fo+1)*SC] = two whole k-subrows.
            def wo_sl(m, fo):
                return wpk[:, 3, 4 * m + 2 * fo : 4 * m + 2 * fo + 2, :]
            bq = cpool.tile([128, 2, 1], f32)
            bk = cpool.tile([128, 2, 1], f32)
            msk = cpool.tile([128, 128], bf16)
            ones_b = cpool.tile([1, 128], bf16)
            nbias = cpool.tile([128, 1], f32)
            qT = cpool.tile([128, 2, S], f16)
            kT = cpool.tile([128, 2, S], f16)
            v1 = cpool.tile([128, S // 128, HPC, D + 1], bf16)
            aoT = cpool.tile([128, 2, S], f16)

            # x tensors load in chunk PAIRS ([128, k, 1024] per k-slice =
            # 2 KB DRAM lines -> ~2x the DMA packet efficiency of 1 KB).
            xfile, xtile = {}, {}

            def emit_dma(pair):
                s0 = pair * 2 * SC
                xf = xfpool.tile([128, 8, 2 * SC], f16, tag="xf", name=f"xf{pair}")
                xt = xtpool.tile([128, 8, 2 * SC], f16, tag="xt", name=f"xt{pair}")
                for sc in (2 * pair, 2 * pair + 1):
                    o = (sc % 2) * SC
                    xfile[sc] = xf[:, :, o : o + SC]
                    xtile[sc] = xt[:, :, o : o + SC]
                for k in range(8):
                    nc.sync.dma_start(
                        out=xf[:, k, :],
                        in_=xf_d[128 * k : 128 * (k + 1), s0 : s0 + 2 * SC],
                    )
                    nc.sync.dma_start(
                        out=xt[:, k, :],
                        in_=xt_d[128 * k : 128 * (k + 1), s0 : s0 + 2 * SC],
                    )

            # startup: weight-blob sections interleaved with the first x
            # loads so the first q-projection matmul starts after ~0.5 MB.
            nc.sync.dma_start(out=bq[:], in_=bq_d[:].rearrange("(m p) c -> p m c", p=128))
            nc.sync.dma_start(out=bk[:], in_=bk_d[:].rearrange("(m p) c -> p m c", p=128))
            nc.sync.dma_start(out=msk[:], in_=msk_d[:])
            nc.vector.memset(ones_b[:], 1.0)
            nc.vector.memset(nbias[:], -12.0)
            # ones column of v1 (col D) -> row D of pav = softmax denominator
            nc.vector.memset(v1[:, :, :, D : D + 1], 1.0)
            xf0 = xfpool.tile([128, 8, 2 * SC], f16, tag="xf", name="xf0")
            xt0 = xtpool.tile([128, 8, 2 * SC], f16, tag="xt", name="xt0")
            for sc in (0, 1):
                o = sc * SC
                xfile[sc] = xf0[:, :, o : o + SC]
                xtile[sc] = xt0[:, :, o : o + SC]
            wpk0 = wpk_d[:, 0, :].rearrange("p (k c) -> p k c", c=CW)
            nc.sync.dma_start(out=wpk[:, 0, 0:4, :], in_=wpk0[:, 0:4, :])
            nc.sync.dma_start(out=xf0[:, 0, :], in_=xf_d[0:128, 0 : 2 * SC])
            nc.sync.dma_start(out=wpk[:, 0, 4:8, :], in_=wpk0[:, 4:8, :])
            for k in range(1, 8):
                nc.sync.dma_start(out=xf0[:, k, :], in_=xf_d[128 * k : 128 * (k + 1), 0 : 2 * SC])
            nc.sync.dma_start(out=wpk[:, 1, :, :], in_=wpk_d[:, 1, :].rearrange("p (k c) -> p k c", c=CW))
            for k in range(8):
                nc.sync.dma_start(out=xt0[:, k, :], in_=xt_d[128 * k : 128 * (k + 1), 0 : 2 * SC])
            nc.sync.dma_start(out=wpk[:, 2, :, :], in_=wpk_d[:, 2, :].rearrange("p (k c) -> p k c", c=CW))
            nc.sync.dma_start(out=wpk[:, 3, :, :], in_=wpk_d[:, 3, :].rearrange("p (k c) -> p k c", c=CW))

            # ---------- projection / out-projection units ----------
            # Each unit takes one [128, 2, SC] slot from the shared PSUM
            # pool (also used by attention score tiles) and is emitted
            # between attention superpairs as PE filler.

            def unit_qk(sc, which):
                s0 = sc * SC
                w, bias, dstT = (
                    (wq, bq, qT) if which == "q" else (wk, bk, kT)
                )
                x = xfile[sc] if which == "q" else xtile[sc]
                pu = ps_u.tile([128, 2, SC], f32, tag="u", name=f"p{which}{sc}")
                for m in range(2):
                    for k in range(8):
                        nc.tensor.matmul(
                            pu[:, m, :],
                            w[:, k, m * 128 : (m + 1) * 128],
                            x[:, k, :],
                            start=(k == 0),
                            stop=(k == 7),
                        )
                for m in range(2):
                    nc.vector.tensor_scalar_add(
                        dstT[:, m, s0 : s0 + SC], pu[:, m, :], bias[:, m, :]
                    )

            def unit_v(sc, vp):
                # NOTE: b_kv[F:] (the v bias) is NOT added here — softmax
                # weights sum to 1, so its contribution to the output is the
                # constant vector bv @ w_out, which the host adds.
                pu = ps_u.tile([128, 2, SC], f32, tag="u", name=f"pv{sc}_{vp}")
                for j in range(2):
                    zz = 2 * vp + j
                    for k in range(8):
                        nc.tensor.matmul(
                            pu[:, j, 0:CW],
                            xtile[sc][:, k, zz * 128 : (zz + 1) * 128],
                            wv[:, k, :],
                            start=(k == 0),
                            stop=(k == 7),
                        )
                for j in range(2):
                    zc = sc * 4 + 2 * vp + j
                    nc.vector.tensor_copy(
                        v1[:, zc, :, 0:D],
                        pu[:, j, 0:CW].rearrange("p (h d) -> p h d", h=HPC),
                    )

            def unit_out(sc, so):
                s0o = sc * SC + so * 128
                pu = ps_u.tile([128, 2, SC], f32, tag="u", name=f"po{sc}_{so}")
                for fo in range(2):
                    for m in range(2):
                        nc.tensor.matmul(
                            pu[:, fo, :],
                            aoT[:, m, s0o : s0o + 128],
                            wo_sl(m, fo),
                            start=(m == 0),
                            stop=(m == 1),
                        )
                ost = opool.tile([128, F], f16, tag="ost")
                nc.vector.tensor_copy(
                    ost[:].rearrange("p (a b) -> p a b", a=2), pu[:, :, :]
                )
                nc.sync.dma_start(out=out_d[s0o : s0o + 128, :], in_=ost[:])

            # chunk-0 projections as a dense prologue
            unit_qk(0, "q")
            unit_qk(0, "k")
            unit_v(0, 0)
            unit_v(0, 1)

            # ---------- flat attention stream ----------
            # pending AVs: deque of closures; AV(sp) is emitted after
            # exp(sp+2) so ACT has two superpairs of slack. Norm tails are
            # due-scheduled two superpairs after their group's last AV.
            av_q = []
            AV_LAG = 2
            # First superpair slot for out-proj filler: the previous
            # chunk's deferred norm flushes at sp AV_LAG+1, and flush runs
            # before fillers within an iteration, so AV_LAG+1 is safe.
            FILL0 = AV_LAG + 1
            pending_norm = []  # (due_gsp, closure)
            gsp = [0]

            def flush_norm(force=False):
                while pending_norm and (force or pending_norm[0][0] <= gsp[0]):
                    pending_norm.pop(0)[1]()

            def drain_av(keep):
                while len(av_q) > keep:
                    av_q.pop(0)()

            for sc in range(NSC):
                s0 = sc * SC
                npair = 2 * (sc + 1)
                nsp = 2 * npair

                # Filler budget per chunk is matched to that chunk's ACT
                # slack: chunk3 (longest attention, no next-chunk proj) gets
                # two out-projection chunks; out(3) runs after the loop.
                # Projection units for the next chunk can fill anywhere; the
                # out-projection units read aoT, so they must wait until the
                # previous chunk's deferred norm has been flushed (FILL0) —
                # Tile tracks access history incrementally in emission order.
                # This chunk's own v-projection units run at superpairs
                # 0-1 (their v1 blocks are first read by AV around superpair
                # 6) — they are the only legal PE filler at the chunk
                # boundary, where ACT lag otherwise starves the PE and
                # re-throttles the clock gate.
                filler = []
                if sc + 1 < NSC:
                    if sc == 0:
                        filler.append(lambda: emit_dma(1))
                    filler.append(lambda sc=sc: unit_qk(sc + 1, "q"))
                    filler.append(lambda sc=sc: unit_qk(sc + 1, "k"))
                self_filler = []
                if sc >= 1:
                    self_filler.append(lambda sc=sc: unit_v(sc, 0))
                    self_filler.append(lambda sc=sc: unit_v(sc, 1))
                late_filler = []
                out_chunks = {2: [0], 3: [1, 2]}.get(sc, [])
                for oc in out_chunks:
                    for so in range(SC // 128):
                        late_filler.append(lambda oc=oc, so=so: unit_out(oc, so))
                import collections as _c

                fill_at = _c.defaultdict(list)
                for u in range(len(self_filler)):
                    fill_at[min(nsp - 1, u)].append(("s", u))
                for u in range(len(filler)):
                    fill_at[min(nsp - 1, 1 + int(u * (nsp - 1) / len(filler)))].append(
                        ("f", u)
                    )
                f0 = min(FILL0, max(0, nsp - 1))
                for u in range(len(late_filler)):
                    fill_at[
                        min(nsp - 1, f0 + int(u * (nsp - f0) / len(late_filler)))
                    ].append(("l", u))

                # dz: first valid s-column of z-block (2p+j); scores/AV skip
                # columns left of it (fully masked). Only a [128,128]
                # staircase band at [dz, dz+128) mixes masked/unmasked.
                def dz_of(p, j, sc=sc):
                    if p < 2 * sc:
                        return -1
                    return 256 * (p - 2 * sc) + 128 * j

                sp_idx = 0
                for m in range(2):
                    pav = [
                        ps_av.tile(
                            [D + 1, SC], f32, tag="pav", name=f"pav{sc}_{m}_{hh}"
                        )
                        for hh in range(2)
                    ]

                    def make_av(p, pt, pav=pav, m=m, npair=npair, dz=dz_of):
                        def emit():
                            for hh in range(2):
                                for j in range(2):
                                    d0 = max(0, dz(p, j))
                                    nc.tensor.matmul(
                                        pav[hh][:, d0:SC],
                                        v1[:, 2 * p + j, 2 * m + hh, :],
                                        pt[hh][:, j, d0:SC],
                                        start=(p == 0 and j == 0),
                                        stop=(p == npair - 1 and j == 1),
                                    )

                        return emit

                    def make_evac(pav=pav, m=m, sc=sc):
                        # Evacuate pav to SBUF right after the group's last
                        # AV: frees the 2 pav banks for the next group ~1us
                        # after the group ends, instead of after the whole
                        # normalization chain.
                        avs = [
                            wpool.tile(
                                [D + 1, SC],
                                f32,
                                tag="avs",
                                name=f"avs{sc}_{m}_{hh}",
                            )
                            for hh in range(2)
                        ]

                        def emit():
                            for hh in range(2):
                                nc.vector.tensor_copy(avs[hh][:, :], pav[hh][:, :])

                        return avs, emit

                    def make_recip(avs, m=m, sc=sc):
                        # 1/den as exp(-ln(den)) on ACT (bf16 out); hoisted
                        # on the saturated ACT queue so it runs as soon as
                        # the denominators land in SBUF.
                        rcbs = [
                            wpool.tile(
                                [1, SC], bf16, tag="rcb", name=f"rcb{sc}_{m}_{hh}"
                            )
                            for hh in range(2)
                        ]

                        def emit():
                            for hh in range(2):
                                lg = wpool.tile(
                                    [1, SC], f32, tag="lg", name=f"lg{sc}_{m}_{hh}"
                                )
                                with tc.high_priority(offset=25):
                                    nc.scalar.activation(
                                        lg[:, :],
                                        avs[hh][D : D + 1, :],
                                        mybir.ActivationFunctionType.Ln,
                                    )
                                    nc.scalar.activation(
                                        rcbs[hh][:, :],
                                        lg[:, :],
                                        mybir.ActivationFunctionType.Exp,
                                        scale=-1.0,
                                    )

                        return rcbs, emit

                    def make_norm(avs, rcbs, m=m, s0=s0, sc=sc):
                        def emit():
                            # PE broadcast of 1/den via ones lhsT, scale into
                            # aoT on DVE. den==0 (row 0) -> garbage only in
                            # column s=0, overwritten by the host.
                            for hh in range(2):
                                po = hh * D
                                pb = ps_u.tile(
                                    [D, SC], f32, tag="u", name=f"pb{sc}_{m}_{hh}"
                                )
                                nc.tensor.matmul(
                                    pb[:, :],
                                    ones_b[:, :D],
                                    rcbs[hh][:, :],
                                    start=True,
                                    stop=True,
                                )
                                sb = wpool.tile(
                                    [D, SC], bf16, tag="sb", name=f"sb{sc}_{m}_{hh}"
                                )
                                nc.vector.tensor_copy(sb[:, :], pb[:, :])
                                nc.vector.tensor_tensor(
                                    out=aoT[po : po + D, m, s0 : s0 + SC],
                                    in0=avs[hh][0:D, :],
                                    in1=sb[:, :],
                                    op=mybir.AluOpType.mult,
                                )

                        return emit

                    for p in range(npair):
                        psp = []
                        pt = []
                        for hh in range(2):
                            po = hh * D
                            ps_h = ps_u.tile(
                                [128, 2, SC],
                                f32,
                                tag="u",
                                name=f"ps{sc}_{m}_{p}_{hh}",
                            )
                            psp.append(ps_h)
                            for j in range(2):
                                z0 = (2 * p + j) * 128
                                d0 = max(0, dz_of(p, j))
                                nc.tensor.matmul(
                                    ps_h[:, j, d0:SC],
                                    kT[po : po + D, m, z0 : z0 + 128],
                                    qT[po : po + D, m, s0 + d0 : s0 + SC],
                                    start=True,
                                    stop=True,
                                )
                        # -12 shift: softmax-invariant, keeps den=sum(exp) in
                        # [2^-46, 2^58] — the ACT Ln table used for 1/den
                        # breaks beyond ~2^63 (logits ~51 -> den ~2^75).
                        # Deep-diagonal pairs (d0 >= 256) are cheaper as two
                        # sliced exps than one full-width one.
                        for hh in range(2):
                            pt_h = ppool.tile(
                                [128, 2, SC],
                                bf16,
                                tag="p",
                                name=f"pt{sc}_{m}_{p}_{hh}",
                            )
                            pt.append(pt_h)
                            if dz_of(p, 0) >= 256:
                                for j in range(2):
                                    d0 = dz_of(p, j)
                                    nc.scalar.activation(
                                        pt_h[:, j, d0:SC],
                                        psp[hh][:, j, d0:SC],
                                        mybir.ActivationFunctionType.Exp,
                                        bias=nbias[:, :],
                                    )
                            else:
                                nc.scalar.activation(
                                    pt_h[:, :, :],
                                    psp[hh][:, :, :],
                                    mybir.ActivationFunctionType.Exp,
                                    bias=nbias[:, :],
                                )
                        for hh in range(2):
                            for j in range(2):
                                d0 = dz_of(p, j)
                                if d0 >= 0:
                                    nc.vector.tensor_tensor(
                                        out=pt[hh][:, j, d0 : d0 + 128],
                                        in0=pt[hh][:, j, d0 : d0 + 128],
                                        in1=msk[:, :],
                                        op=mybir.AluOpType.mult,
                                    )
                        av_q.append(make_av(p, pt))
                        drain_av(AV_LAG)
                        flush_norm()
                        for kind, u in fill_at.get(sp_idx, ()):
                            {"f": filler, "l": late_filler, "s": self_filler}[
                                kind
                            ][u]()
                        sp_idx += 1
                        gsp[0] += 1
                    # The evac depends on this group's last AV, which drains
                    # AV_LAG-1 superpairs from now (drain runs before flush in
                    # the loop). Both it and the norm MUST be emitted before
                    # anything that reads aoT or reuses the pav banks: Tile
                    # tracks access history incrementally in emission order,
                    # so a reader emitted after an overwriting write is NOT
                    # protected.
                    avs, evac = make_evac()
                    rcbs, recip = make_recip(avs)
                    pending_norm.append((gsp[0] + AV_LAG - 1, evac))
                    pending_norm.append((gsp[0] + AV_LAG, recip))
                    pending_norm.append((gsp[0] + AV_LAG + 1, make_norm(avs, rcbs)))

            drain_av(0)
            flush_norm(force=True)
            for so in range(SC // 128):
                unit_out(NSC - 1, so)

    _split_excess_waits(nc)
    return nc


_CACHE = {}


def _get_nc():
    if "nc" not in _CACHE:
        _CACHE["nc"] = _build()
    return _CACHE["nc"]


def _ensure_ntff_hook():
    """The agent image's antenv lacks axon_hooks, so run_bass_kernel_spmd's
    trace path can't import it. Synthesize the module and install the
    ctypes NTFF hook from trn_agent_boot (same thing boot() would do)."""
    import sys
    import types

    if "antenv.axon_hooks" not in sys.modules:
        mod = types.ModuleType("antenv.axon_hooks")
        holder = [None]
        mod.set_axon_ntff_profile_hook = lambda h: holder.__setitem__(0, h)
        mod.get_axon_ntff_profile_hook = lambda: holder[0]
        sys.modules["antenv.axon_hooks"] = mod
        import antenv

        antenv.axon_hooks = mod
    import antenv.axon_hooks as ah

    if ah.get_axon_ntff_profile_hook() is None:
        try:
            from trn_agent_boot.trn_boot import _ntff_profile_via_ctypes

            ah.set_axon_ntff_profile_hook(
                _ntff_profile_via_ctypes("/opt/axon/libaxon_pjrt.so")
            )
        except Exception:
            pass


def _host_mask():
    # The causal band of every diagonal score block (z0 = s0 + dz) reduced to
    # its mixed [dz, dz+128) columns is the same staircase: element
    # (z = z0 + i, s = s0 + dz + c) is masked iff s <= z iff c <= i. Applied
    # multiplicatively to exp(scores) in bf16.
    i = np.arange(128)[:, None]
    c = np.arange(128)[None, :]
    import jax.numpy as _jnp
    m = np.where(c <= i, 0.0, 1.0).astype(np.float32)
    return np.asarray(_jnp.asarray(m, dtype=_jnp.bfloat16))


def kernel(attend_from, attend_to, w_q, b_q, w_kv, b_kv, w_out, b_out, _trace=False):
    attend_from = np.asarray(attend_from, dtype=np.float32)
    attend_to = np.asarray(attend_to, dtype=np.float32)
    w_q = np.asarray(w_q, dtype=np.float32)
    b_q = np.asarray(b_q, dtype=np.float32)
    w_kv = np.asarray(w_kv, dtype=np.float32)
    b_kv = np.asarray(b_kv, dtype=np.float32)
    w_out = np.asarray(w_out, dtype=np.float32)
    b_out = np.asarray(b_out, dtype=np.float32)

    msk = _host_mask()
    xT = [attend_from[b].T.astype(np.float16) for b in range(B)]
    xTt = [attend_to[b].T.astype(np.float16) for b in range(B)]

    def kmajor(w):
        # [F, CW] -> [128, F//128 * CW]: row p = concat_k w[k*128+p, :]
        return np.ascontiguousarray(
            w.reshape(8, 128, CW).transpose(1, 0, 2).reshape(128, 8 * CW)
        )

    in_maps = []
    for c in range(NCORES):
        b, hg = divmod(c, HG)
        cols = slice(hg * CW, (hg + 1) * CW)
        # wo section: [CW, F] m-major -> [128, 2*F]: row p =
        # concat_m w_out[m*128+p, :]
        wo_pk = (
            w_out[cols, :]
            .reshape(2, 128, F)
            .transpose(1, 0, 2)
            .reshape(128, 2 * F)
        )
        wpk = np.stack(
            [
                kmajor(w_q[:, cols]),
                kmajor(w_kv[:, cols]),
                kmajor(w_kv[:, F:][:, cols]),
                wo_pk,
            ],
            axis=1,
        ).astype(np.float16)
        in_maps.append(
            {
                "xf": xT[b],
                "xt": xTt[b],
                "wpk": np.ascontiguousarray(wpk),
                "bq": np.ascontiguousarray(b_q[cols].reshape(CW, 1)),
                "bk": np.ascontiguousarray(b_kv[cols].reshape(CW, 1)),
                "msk": msk,
                "out": np.zeros((S, F), np.float16),
            }
        )

    nc = _get_nc()
    if _trace:
        _ensure_ntff_hook()
    res = run_bass_kernel_spmd(nc, in_maps, list(range(NCORES)), trace=_trace)

    out = np.zeros((B, S, F), np.float64)
    for c in range(NCORES):
        b = c // HG
        out[b] += res.results[c]["out"].astype(np.float64)
    # v-bias contribution: softmax rows sum to 1, so it collapses to the
    # constant vector b_v @ w_out (the kernel omits it)
    out += (
        b_out.astype(np.float64)
        + b_kv[F:].astype(np.float64) @ w_out.astype(np.float64)
    )[None, None, :]

    # Row 0 of the reference is fully masked -> softmax is exactly uniform
    # over all Z positions (the -1e12 shift absorbs the logits in f32);
    # compute it directly on the host.
    w_v = w_kv[:, F:].astype(np.float64)
    for b in range(B):
        val_mean = attend_to[b].astype(np.float64).mean(axis=0) @ w_v + b_kv[
            F:
        ].astype(np.float64)
        out[b, 0, :] = val_mean @ w_out.astype(np.float64) + b_out.astype(np.float64)

    if _trace:
        kernel._last_result = res
    return out.astype(np.float32)
